# revision 8
# baseline (speedup 1.0000x reference)
"""Trainium2 Bass kernel for BoundingBoxRegressorAndMultiLabelClassifier.

v2: interp is data-parallel (2 samples/core); the big roi_fc is K-sharded by
channel across the 8 cores (6.4MB of weights per core instead of 51.4MB),
stitched with an AllToAll of pooled features and a ReduceScatter of the
partial FC products. Heads run per-core on the 52 rows each core owns.

ROI align is a joint matmul: pooled[c,(p,q,n)] = sum_{hw} feat^T[hw,c] *
M^T[hw,(p,q,n)], with M^T built on host from box coords (index math only).
Matmuls use float32r (fp32 w/ 11-bit mantissa, ~1.5 cyc/row on PE).
"""
import numpy as np
import concourse.bass as bass
import concourse.tile as tile
from concourse import bacc, mybir
from concourse import bass_utils

# ---- problem constants (hardcoded per contract) ----
B, C, H = 16, 512, 32
HID, NB, S, P, SR = 512, 36, 26, 7, 2
SCALE = 32.0
N_LOC, N_LOC_LAB = 12, 8
N_GRP, GRP_SZ, N_GRP_LAB = 4, 6, 16

NCORES = 8
SPC = B // NCORES          # samples per core = 2
PQ = P * P                 # 49
NPQ = S * PQ               # 1274 (columns ordered (p,q,n) pq-major!)
CCH = C // 128             # 4 channel chunks (interp output)
KCH = (H * H) // 128       # 8 hw chunks
MROWS = SPC * S            # 52 rows per core (heads)
MALL = B * S               # 416 rows total (FC)
CLOC = C // NCORES         # 64 channels per core (FC K-shard)
NKJ = 25                   # FC K-chunks: 24 of (64c x 2pq) + 1 of (64c x 1pq)
NCHUNKS = [(0, 512), (512, 512), (1024, NPQ - 1024)]
MCH = [(0, 128), (128, 128), (256, 128), (384, MALL - 384)]

f32 = mybir.dt.float32
f32r = mybir.dt.float32r


def _r(x: np.ndarray) -> np.ndarray:
    """RNE-round fp32 to fp32r (11-bit mantissa) so host data matches what the
    PE consumes; keeps sim == hw."""
    b = np.ascontiguousarray(x, dtype=np.float32).view(np.uint32)
    r = b + np.uint32(0x7FF) + ((b >> np.uint32(12)) & np.uint32(1))
    return (r & np.uint32(0xFFFFF000)).view(np.float32)


def _interp_mats(coords: np.ndarray):
    """coords (B,S,4) -> Qy, Qx (B,S,P,H) fp32, pooling+validity folded in."""
    c = coords.astype(np.float32)
    x1 = c[..., 0] * np.float32(SCALE)
    y1 = c[..., 1] * np.float32(SCALE)
    x2 = c[..., 2] * np.float32(SCALE)
    y2 = c[..., 3] * np.float32(SCALE)
    rw = np.maximum(x2 - x1, np.float32(1.0))
    rh = np.maximum(y2 - y1, np.float32(1.0))
    bw = rw / np.float32(P)
    bh = rh / np.float32(P)
    off = ((np.arange(P, dtype=np.float32)[:, None]
            + (np.arange(SR, dtype=np.float32) + np.float32(0.5)) / np.float32(SR))
           .reshape(-1))                                   # (P*SR,)
    ys = y1[..., None] + off * bh[..., None]               # (B,S,14)
    xs = x1[..., None] + off * bw[..., None]

    eye = np.eye(H, dtype=np.float32)

    def qmat(t):
        valid = ((t > -1.0) & (t < H)).astype(np.float32)
        tc = np.clip(t, np.float32(0.0), np.float32(H - 1))
        lo = np.floor(tc).astype(np.int64)
        hi = np.minimum(lo + 1, H - 1)
        fr = (tc - lo.astype(np.float32)).astype(np.float32)
        R = (eye[lo] * ((1.0 - fr) * valid)[..., None]
             + eye[hi] * (fr * valid)[..., None])          # (B,S,14,H)
        return R.reshape(B, S, P, SR, H).mean(axis=3)      # (B,S,P,H)

    return qmat(ys), qmat(xs)


_BUILT = None


def _build_program():
    nc = bacc.Bacc("TRN2", target_bir_lowering=False, debug=False,
                   enable_asserts=False, num_devices=NCORES)

    d = {}
    def din(name, shape, dt=f32r):
        d[name] = nc.dram_tensor(name, list(shape), dt, kind="ExternalInput").ap()
    def dout(name, shape, dt=f32):
        d[name] = nc.dram_tensor(name, list(shape), dt, kind="ExternalOutput").ap()

    din("featT", (SPC, H * H, C))
    din("MT", (SPC, H * H, NPQ))
    din("Wsl", (NKJ, 128, HID))          # per-core FC weight K-slice
    din("fcbias8", (1, HID))             # roi_fc_b / 8 (pre-reduce trick)
    din("ones", (1, 128))
    din("ident", (128, 128), f32)
    din("cwT", (HID, S * 4))
    din("pwT", (HID, S))
    din("lwT", (HID, N_LOC * N_LOC_LAB))
    din("gwT", (GRP_SZ, HID, N_GRP * N_GRP_LAB))
    din("maskC", (S * 4, MROWS), f32)
    din("maskP", (S, MROWS), f32)
    din("maskL", (N_LOC * N_LOC_LAB, MROWS), f32)
    din("maskG", (N_GRP * N_GRP_LAB, GRP_SZ * SPC * N_GRP), f32)
    din("cbias", (S * 4, 1), f32)
    din("pbias", (S, 1), f32)
    din("lbias", (N_LOC * N_LOC_LAB, 1), f32)
    din("gbias", (N_GRP * N_GRP_LAB, 1), f32)
    din("predT", (S * 4, SPC), f32)
    din("whT", (S * 4, SPC), f32)
    dout("o_ref", (S * 4, SPC))
    dout("o_pres", (S, SPC))
    dout("o_loc", (N_LOC * N_LOC_LAB, SPC))
    dout("o_grp", (N_GRP * N_GRP_LAB, SPC))

    NLOCR = N_LOC * N_LOC_LAB   # 96
    NGRPR = N_GRP * N_GRP_LAB   # 64
    PAY = SPC * NPQ             # 2548 per (src, c_local) payload

    with tile.TileContext(nc) as tc:
        with (tc.tile_pool(name="const", bufs=1) as cp,
              tc.tile_pool(name="big", bufs=1) as bp,
              tc.tile_pool(name="wst", bufs=3) as wp,
              tc.tile_pool(name="lh", bufs=3) as lp,
              tc.tile_pool(name="wk", bufs=2) as wk,
              tc.tile_pool(name="dram", bufs=1, space="DRAM") as dp,
              tc.tile_pool(name="psI", bufs=2, space="PSUM") as psI,
              tc.tile_pool(name="psF", bufs=1, space="PSUM") as psF,
              tc.tile_pool(name="psT", bufs=1, space="PSUM") as psT,
              tc.tile_pool(name="psH", bufs=1, space="PSUM") as psH):

            # ---- persistent loads ----
            featT = bp.tile([128, SPC * KCH * C], f32r, tag="featT")
            MT = bp.tile([128, SPC * KCH * NPQ], f32r, tag="MT")
            for s in range(SPC):
                for k in range(KCH):
                    nc.sync.dma_start(
                        featT[:, (s * KCH + k) * C:(s * KCH + k + 1) * C],
                        d["featT"][s, k * 128:(k + 1) * 128, :])
                    nc.sync.dma_start(
                        MT[:, (s * KCH + k) * NPQ:(s * KCH + k + 1) * NPQ],
                        d["MT"][s, k * 128:(k + 1) * 128, :])

            def cload(name, shape, dt=f32r):
                t = cp.tile(list(shape), dt, tag=name, name=name + "_sb")
                nc.sync.dma_start(t[:], d[name][:])
                return t
            ones_sb = cload("ones", (1, 128))
            fcb_sb = cload("fcbias8", (1, HID))
            id_sb = cload("ident", (128, 128), f32)
            maskC = cload("maskC", (S * 4, MROWS), f32)
            maskP = cload("maskP", (S, MROWS), f32)
            maskL = cload("maskL", (NLOCR, MROWS), f32)
            maskG = cload("maskG", (NGRPR, GRP_SZ * SPC * N_GRP), f32)
            cbias = cload("cbias", (S * 4, 1), f32)
            pbias = cload("pbias", (S, 1), f32)
            lbias = cload("lbias", (NLOCR, 1), f32)
            gbias = cload("gbias", (NGRPR, 1), f32)
            predT = cload("predT", (S * 4, SPC), f32)
            whT = cload("whT", (S * 4, SPC), f32)

            cwT = cp.tile([128, CCH * S * 4], f32r, tag="cwT")
            pwT = cp.tile([128, CCH * S], f32r, tag="pwT")
            lwT = cp.tile([128, CCH * NLOCR], f32r, tag="lwT")
            gwT = cp.tile([128, GRP_SZ * CCH * NGRPR], f32r, tag="gwT")
            for hc in range(CCH):
                nc.sync.dma_start(cwT[:, hc * S * 4:(hc + 1) * S * 4],
                                  d["cwT"][hc * 128:(hc + 1) * 128, :])
                nc.sync.dma_start(pwT[:, hc * S:(hc + 1) * S],
                                  d["pwT"][hc * 128:(hc + 1) * 128, :])
                nc.sync.dma_start(lwT[:, hc * NLOCR:(hc + 1) * NLOCR],
                                  d["lwT"][hc * 128:(hc + 1) * 128, :])
                for m in range(GRP_SZ):
                    nc.sync.dma_start(
                        gwT[:, (m * CCH + hc) * NGRPR:(m * CCH + hc + 1) * NGRPR],
                        d["gwT"][m, hc * 128:(hc + 1) * 128, :])

            # ---- ROI-align interp as joint matmuls ----
            pooled = [bp.tile([128, PAY], f32r, tag=f"pooled{cc}",
                              name=f"pooled{cc}")
                      for cc in range(CCH)]
            for s in range(SPC):
                for cc in range(CCH):
                    for (n0, nn) in NCHUNKS:
                        pt = psI.tile([128, nn], f32, tag="psI", name="ptI")
                        for k in range(KCH):
                            base = (s * KCH + k)
                            nc.tensor.matmul(
                                pt[:],
                                featT[:, base * C + cc * 128: base * C + cc * 128 + 128],
                                MT[:, base * NPQ + n0: base * NPQ + n0 + nn],
                                start=(k == 0), stop=(k == KCH - 1))
                        nc.vector.tensor_copy(
                            pooled[cc][:, s * NPQ + n0: s * NPQ + n0 + nn], pt[:])

            # ---- AllToAll: redistribute pooled to channel shards ----
            a2a_in = dp.tile([NCORES, CLOC, PAY], f32r, tag="a2a_in",
                             name="a2a_in")
            a2a_out = dp.tile([NCORES, CLOC, PAY], f32r, tag="a2a_out",
                              name="a2a_out")
            for cc in range(CCH):
                for e in range(2):
                    nc.sync.dma_start(a2a_in[2 * cc + e],
                                      pooled[cc][e * CLOC:(e + 1) * CLOC, :])
            nc.gpsimd.collective_compute(
                "AllToAll", mybir.AluOpType.bypass,
                replica_groups=[list(range(NCORES))],
                ins=[a2a_in.opt()], outs=[a2a_out.opt()])

            # ---- K-sharded roi_fc: partial over this core's 64 channels ----
            # a2a_out[src, c, sl*NPQ + pq*S + n]; K-chunk j = (e,c) with
            # pq = 2j+e. lhsT free = (src, sl, n) = global row order.
            obv = a2a_out.rearrange("src c (sl pqn) -> c src sl pqn", sl=SPC)
            fps = []
            for mc, (m0, mn) in enumerate(MCH):
                ft = psF.tile([mn, HID], f32, tag=f"psF{mc}", name=f"psF{mc}")
                fps.append(ft)
                nc.tensor.matmul(ft[:], ones_sb[:1, :mn], fcb_sb[:],
                                 start=True, stop=False)
            for j in range(NKJ):
                ne = 2 if j < NKJ - 1 else 1
                lt = lp.tile([128, MALL], f32r, tag="lhsT", name="lhsT")
                for e in range(ne):
                    for sl in range(SPC):
                        nc.sync.dma_start(
                            lt[e * CLOC:(e + 1) * CLOC, :]
                            .rearrange("p (src sl n) -> p src sl n",
                                       src=NCORES, sl=SPC)[:, :, sl, :],
                            obv[:, :, sl, (2 * j + e) * S:(2 * j + e) * S + S])
                wt = wp.tile([128, HID], f32r, tag="wst", name="wt")
                nc.sync.dma_start(wt[:], d["Wsl"][j])
                kk = ne * CLOC
                for mc, (m0, mn) in enumerate(MCH):
                    nc.tensor.matmul(
                        fps[mc][:], lt[:kk, m0:m0 + mn], wt[:kk, :],
                        start=False, stop=(j == NKJ - 1))

            # ---- ReduceScatter partials -> this core's 52 rows ----
            partial = dp.tile([MALL, HID], f32, tag="partial", name="partial")
            rs_out = dp.tile([MROWS, HID], f32, tag="rs_out", name="rs_out")
            for mc, (m0, mn) in enumerate(MCH):
                pst = wk.tile([mn, HID], f32, tag="pstage", name=f"pstage{mc}")
                nc.vector.tensor_copy(pst[:], fps[mc][:])
                nc.sync.dma_start(partial[m0:m0 + mn, :], pst[:])
            nc.gpsimd.collective_compute(
                "ReduceScatter", mybir.AluOpType.add,
                replica_groups=[list(range(NCORES))],
                ins=[partial.opt()], outs=[rs_out.opt()])

            feats_pre = wk.tile([MROWS, HID], f32, tag="feats_pre")
            nc.sync.dma_start(feats_pre[:], rs_out[:])
            feats = wk.tile([MROWS, HID], f32, tag="feats")
            nc.vector.tensor_relu(feats[:], feats_pre[:])

            # ---- transpose feats -> featsT (h-part, (s,n)-free) ----
            featsT = wk.tile([128, CCH * MROWS], f32r, tag="featsT")
            for hc in range(CCH):
                tp = psT.tile([128, MROWS], f32, tag="psT", name="tpT")
                nc.tensor.transpose(tp[:], feats[:, hc * 128:(hc + 1) * 128],
                                    id_sb[:MROWS, :MROWS])
                nc.vector.tensor_copy(featsT[:, hc * MROWS:(hc + 1) * MROWS], tp[:])

            # ---- heads (matmul + mask + segment-reduce) ----
            def head_mm(wtile, nrows, wstride, name):
                hp = psH.tile([nrows, MROWS], f32, tag="psH", name=name)
                for hc in range(CCH):
                    nc.tensor.matmul(
                        hp[:], wtile[:, hc * wstride: hc * wstride + nrows],
                        featsT[:, hc * MROWS:(hc + 1) * MROWS],
                        start=(hc == 0), stop=(hc == CCH - 1))
                return hp

            def mask_reduce(hp, nrows, mask, tagn):
                mskd = wk.tile([nrows, MROWS], f32, tag="m" + tagn,
                               name="m" + tagn)
                nc.vector.tensor_mul(mskd[:], hp[:], mask[:])
                red = wk.tile([nrows, SPC], f32, tag="r" + tagn,
                              name="r" + tagn)
                nc.vector.reduce_sum(
                    red[:], mskd[:].rearrange("p (s n) -> p s n", s=SPC),
                    axis=mybir.AxisListType.X)
                return red

            hpC = head_mm(cwT, S * 4, S * 4, "hpC")
            redC = mask_reduce(hpC, S * 4, maskC, "C")
            nc.vector.tensor_scalar_add(redC[:], redC[:], cbias[:])
            nc.vector.tensor_mul(redC[:], redC[:], whT[:])
            nc.vector.tensor_add(redC[:], redC[:], predT[:])
            nc.sync.dma_start(d["o_ref"][:], redC[:])

            hpP = head_mm(pwT, S, S, "hpP")
            redP = mask_reduce(hpP, S, maskP, "P")
            nc.vector.tensor_scalar_add(redP[:], redP[:], pbias[:])
            nc.sync.dma_start(d["o_pres"][:], redP[:])

            hpL = head_mm(lwT, NLOCR, NLOCR, "hpL")
            redL = mask_reduce(hpL, NLOCR, maskL, "L")
            nc.vector.tensor_scalar_add(redL[:], redL[:], lbias[:])
            nc.sync.dma_start(d["o_loc"][:], redL[:])

            gp = psH.tile([NGRPR, GRP_SZ * SPC * N_GRP], f32, tag="psH",
                          name="gp")
            for m in range(GRP_SZ):
                for hc in range(CCH):
                    rhsv = (featsT[:, hc * MROWS:(hc + 1) * MROWS]
                            .rearrange("p (s n) -> p s n", s=SPC)
                            [:, :, m:m + (N_GRP - 1) * GRP_SZ + 1:GRP_SZ])
                    nc.tensor.matmul(
                        gp[:, m * SPC * N_GRP:(m + 1) * SPC * N_GRP],
                        gwT[:, (m * CCH + hc) * NGRPR:(m * CCH + hc + 1) * NGRPR],
                        rhsv,
                        start=(hc == 0), stop=(hc == CCH - 1))
            mskdG = wk.tile([NGRPR, GRP_SZ * SPC * N_GRP], f32, tag="mG")
            nc.vector.tensor_mul(mskdG[:], gp[:], maskG[:])
            r1 = wk.tile([NGRPR, GRP_SZ * SPC], f32, tag="r1G")
            nc.vector.reduce_sum(
                r1[:], mskdG[:].rearrange("p (m s g) -> p m s g", s=SPC, g=N_GRP),
                axis=mybir.AxisListType.X)
            redG = wk.tile([NGRPR, SPC], f32, tag="rG")
            nc.vector.reduce_sum(
                redG[:], r1[:].rearrange("p (m s) -> p s m", s=SPC),
                axis=mybir.AxisListType.X)
            nc.vector.tensor_scalar_add(redG[:], redG[:], gbias[:])
            nc.sync.dma_start(d["o_grp"][:], redG[:])

    nc.compile()
    return nc


def kernel(**inputs) -> tuple:
    global _BUILT
    if _BUILT is None:
        _BUILT = _build_program()
    nc = _BUILT

    lf = np.asarray(inputs["local_features"], np.float32)
    coords = np.asarray(inputs["pred_bbox_coords"], np.float32)
    Wfc = np.asarray(inputs["roi_fc_W"], np.float32)
    fcb = np.asarray(inputs["roi_fc_b"], np.float32)
    coords_W = np.asarray(inputs["coords_W"], np.float32)
    coords_b = np.asarray(inputs["coords_b"], np.float32)
    pres_W = np.asarray(inputs["pres_W"], np.float32)
    pres_b = np.asarray(inputs["pres_b"], np.float32)
    loc_W = np.asarray(inputs["loc_W"], np.float32)
    loc_b = np.asarray(inputs["loc_b"], np.float32)
    grp_W = np.asarray(inputs["grp_W"], np.float32)
    grp_b = np.asarray(inputs["grp_b"], np.float32)
    loc_idx = np.asarray(inputs["loc_idx"], np.int64)
    grp_idx = np.asarray(inputs["grp_idx"], np.int64)

    # ---- host prep ----
    Qy, Qx = _interp_mats(coords)
    # MT[b] (1024, NPQ), columns (p,q,n) pq-major
    MT = np.einsum("bnph,bnqw->bhwpqn", Qy, Qx).reshape(B, H * H, NPQ)
    featT = lf.transpose(0, 2, 3, 1).reshape(B, H * H, C)

    # per-core FC weight K-slices: chunk j rows (e*64+c) = Wfc col
    # ((64k+c)*PQ + 2j+e); last chunk zero-padded to 128 rows
    Wr = Wfc.reshape(HID, C, PQ)

    cwT = coords_W.transpose(2, 0, 1).reshape(HID, S * 4)
    pwT = pres_W.T
    lwT = loc_W.transpose(2, 0, 1).reshape(HID, N_LOC * N_LOC_LAB)
    gw4 = grp_W.reshape(N_GRP, N_GRP_LAB, GRP_SZ, HID)
    gwT = np.stack([gw4[:, :, m, :].transpose(2, 0, 1)
                    .reshape(HID, N_GRP * N_GRP_LAB) for m in range(GRP_SZ)])

    maskC = np.zeros((S * 4, MROWS), np.float32)
    for n in range(S):
        for o in range(4):
            for s in range(SPC):
                maskC[n * 4 + o, s * S + n] = 1.0
    maskP = np.zeros((S, MROWS), np.float32)
    for n in range(S):
        for s in range(SPC):
            maskP[n, s * S + n] = 1.0
    maskL = np.zeros((N_LOC * N_LOC_LAB, MROWS), np.float32)
    for l in range(N_LOC):
        for o in range(N_LOC_LAB):
            for s in range(SPC):
                maskL[l * N_LOC_LAB + o, s * S + int(loc_idx[l])] = 1.0
    exp_gidx = np.arange(N_GRP * GRP_SZ).reshape(N_GRP, GRP_SZ)
    assert np.array_equal(grp_idx, exp_gidx), "grp_idx pattern unsupported"
    maskG = np.zeros((N_GRP * N_GRP_LAB, GRP_SZ * SPC * N_GRP), np.float32)
    for g in range(N_GRP):
        for o in range(N_GRP_LAB):
            for m in range(GRP_SZ):
                for s in range(SPC):
                    maskG[g * N_GRP_LAB + o, m * SPC * N_GRP + s * N_GRP + g] = 1.0

    shared = {
        "fcbias8": _r(fcb.reshape(1, HID) / np.float32(NCORES)),
        "ones": _r(np.ones((1, 128), np.float32)),
        "ident": np.eye(128, dtype=np.float32),
        "cwT": _r(cwT), "pwT": _r(pwT), "lwT": _r(lwT), "gwT": _r(gwT),
        "maskC": maskC, "maskP": maskP, "maskL": maskL, "maskG": maskG,
        "cbias": coords_b.reshape(S * 4, 1).astype(np.float32),
        "pbias": pres_b.reshape(S, 1).astype(np.float32),
        "lbias": loc_b.reshape(N_LOC * N_LOC_LAB, 1).astype(np.float32),
        "gbias": grp_b.reshape(N_GRP * N_GRP_LAB, 1).astype(np.float32),
    }

    w = coords[..., 2] - coords[..., 0]
    h = coords[..., 3] - coords[..., 1]
    wh = np.stack([w, h, w, h], axis=-1)  # (B,S,4)

    in_maps = []
    for k in range(NCORES):
        sl = slice(k * SPC, (k + 1) * SPC)
        # FC K-slice for this core's channels [64k, 64k+64)
        sub = Wr[:, k * CLOC:(k + 1) * CLOC, :]          # (HID, 64, PQ)
        arr = sub.transpose(2, 1, 0)                     # (PQ, 64, HID)
        arr = np.concatenate(
            [arr, np.zeros((1, CLOC, HID), np.float32)], axis=0)  # pad pq=49
        Wsl = np.ascontiguousarray(arr.reshape(NKJ, 128, HID))
        m = dict(shared)
        m["Wsl"] = _r(Wsl)
        m["featT"] = _r(featT[sl])
        m["MT"] = _r(MT[sl])
        m["predT"] = np.ascontiguousarray(
            coords[sl].transpose(1, 2, 0).reshape(S * 4, SPC))
        m["whT"] = np.ascontiguousarray(
            wh[sl].transpose(1, 2, 0).reshape(S * 4, SPC))
        in_maps.append(m)

    res = bass_utils.run_bass_kernel_spmd(nc, in_maps, core_ids=list(range(NCORES)))
    kernel.last_result = res

    refined = np.zeros((B, S, 4), np.float32)
    presence = np.zeros((B, S), np.float32)
    mlc = np.zeros((B, N_LOC * N_LOC_LAB + N_GRP * N_GRP_LAB), np.float32)
    for k in range(NCORES):
        r = res.results[k]
        for s in range(SPC):
            b = k * SPC + s
            refined[b] = r["o_ref"][:, s].reshape(S, 4)
            presence[b] = r["o_pres"][:, s]
            mlc[b, :N_LOC * N_LOC_LAB] = r["o_loc"][:, s]
            mlc[b, N_LOC * N_LOC_LAB:] = r["o_grp"][:, s]
    return refined, presence, mlc


# revision 9
# speedup vs baseline: 1.2841x; 1.2841x over previous
"""Trainium2 Bass kernel for BoundingBoxRegressorAndMultiLabelClassifier.

Pure data-parallel over batch: each of 8 cores runs 2 samples end-to-end.
ROI align is reformulated as a joint matmul per sample:
    pooled[c, (n,p,q)] = sum_{h,w} feat^T[(h,w), c] * M^T[(h,w), (n,p,q)]
where the interpolation matrix M^T is built on host from the box coords
(pure index math); the heavy gather/arithmetic stays on device as dense
matmuls. roi_fc consumes pooled via stride-49 AP views (zero transposes),
with the weight matrix streamed from HBM. The channel-chunk (cc) loop is
outermost so FC k-chunks for cc start as soon as pooled[cc] is ready --
this overlaps the 51MB weight stream with the interp phase.

Matmuls run in float32r (fp32 with 11-bit mantissa, ~1.5 cyc/row at N>=256;
end-to-end output error ~3e-4 scale-relative vs the fp32 reference).
"""
import numpy as np
import concourse.bass as bass
import concourse.tile as tile
from concourse import bacc, mybir
from concourse import bass_utils

# ---- problem constants (hardcoded per contract) ----
B, C, H = 16, 512, 32
HID, NB, S, P, SR = 512, 36, 26, 7, 2
SCALE = 32.0
N_LOC, N_LOC_LAB = 12, 8
N_GRP, GRP_SZ, N_GRP_LAB = 4, 6, 16

NCORES = 8
SPC = B // NCORES          # samples per core = 2
PQ = P * P                 # 49
NPQ = S * PQ               # 1274 (columns (n,p,q), n-major)
CCH = C // 128             # 4 channel chunks
KCH = (H * H) // 128       # 8 hw chunks
MROWS = SPC * S            # 52 box-rows per core
NCHUNKS = [(0, 512), (512, 512), (1024, NPQ - 1024)]
WB = 4                     # pq per weight-stream DMA batch
WBATCH = [(b0, min(WB, PQ - b0)) for b0 in range(0, PQ, WB)]

f32 = mybir.dt.float32
f32r = mybir.dt.float32r


def _r(x: np.ndarray) -> np.ndarray:
    """RNE-round fp32 to fp32r (11-bit mantissa) so host data matches what the
    PE consumes; keeps sim == hw."""
    b = np.ascontiguousarray(x, dtype=np.float32).view(np.uint32)
    r = b + np.uint32(0x7FF) + ((b >> np.uint32(12)) & np.uint32(1))
    return (r & np.uint32(0xFFFFF000)).view(np.float32)


def _interp_mats(coords: np.ndarray):
    """coords (B,S,4) -> Qy, Qx (B,S,P,H) fp32, pooling+validity folded in."""
    c = coords.astype(np.float32)
    x1 = c[..., 0] * np.float32(SCALE)
    y1 = c[..., 1] * np.float32(SCALE)
    x2 = c[..., 2] * np.float32(SCALE)
    y2 = c[..., 3] * np.float32(SCALE)
    rw = np.maximum(x2 - x1, np.float32(1.0))
    rh = np.maximum(y2 - y1, np.float32(1.0))
    bw = rw / np.float32(P)
    bh = rh / np.float32(P)
    off = ((np.arange(P, dtype=np.float32)[:, None]
            + (np.arange(SR, dtype=np.float32) + np.float32(0.5)) / np.float32(SR))
           .reshape(-1))                                   # (P*SR,)
    ys = y1[..., None] + off * bh[..., None]               # (B,S,14)
    xs = x1[..., None] + off * bw[..., None]

    eye = np.eye(H, dtype=np.float32)

    def qmat(t):
        valid = ((t > -1.0) & (t < H)).astype(np.float32)
        tc = np.clip(t, np.float32(0.0), np.float32(H - 1))
        lo = np.floor(tc).astype(np.int64)
        hi = np.minimum(lo + 1, H - 1)
        fr = (tc - lo.astype(np.float32)).astype(np.float32)
        R = (eye[lo] * ((1.0 - fr) * valid)[..., None]
             + eye[hi] * (fr * valid)[..., None])          # (B,S,14,H)
        return R.reshape(B, S, P, SR, H).mean(axis=3)      # (B,S,P,H)

    return qmat(ys), qmat(xs)


_BUILT = None


def _build_program():
    nc = bacc.Bacc("TRN2", target_bir_lowering=False, debug=False,
                   enable_asserts=False, num_devices=NCORES)

    d = {}
    def din(name, shape, dt=f32r):
        d[name] = nc.dram_tensor(name, list(shape), dt, kind="ExternalInput").ap()
    def dout(name, shape, dt=f32):
        d[name] = nc.dram_tensor(name, list(shape), dt, kind="ExternalOutput").ap()

    din("featT", (SPC, H * H, C))
    din("MT", (SPC, H * H, NPQ))
    din("Wst", (CCH, PQ, 128, HID))
    din("fcbias", (1, HID))
    din("ones", (1, 128))
    din("ident", (128, 128), f32)
    din("cwT", (HID, S * 4))
    din("pwT", (HID, S))
    din("lwT", (HID, N_LOC * N_LOC_LAB))
    din("gwT", (GRP_SZ, HID, N_GRP * N_GRP_LAB))
    din("maskC", (S * 4, MROWS), f32)
    din("maskP", (S, MROWS), f32)
    din("maskL", (N_LOC * N_LOC_LAB, MROWS), f32)
    din("maskG", (N_GRP * N_GRP_LAB, GRP_SZ * SPC * N_GRP), f32)
    din("cbias", (S * 4, 1), f32)
    din("pbias", (S, 1), f32)
    din("lbias", (N_LOC * N_LOC_LAB, 1), f32)
    din("gbias", (N_GRP * N_GRP_LAB, 1), f32)
    din("predT", (S * 4, SPC), f32)
    din("whT", (S * 4, SPC), f32)
    dout("o_ref", (S * 4, SPC))
    dout("o_pres", (S, SPC))
    dout("o_loc", (N_LOC * N_LOC_LAB, SPC))
    dout("o_grp", (N_GRP * N_GRP_LAB, SPC))

    NLOCR = N_LOC * N_LOC_LAB   # 96
    NGRPR = N_GRP * N_GRP_LAB   # 64

    with tile.TileContext(nc) as tc:
        with (tc.tile_pool(name="const", bufs=1) as cp,
              tc.tile_pool(name="big", bufs=1) as bp,
              tc.tile_pool(name="wst", bufs=3) as wp,
              tc.tile_pool(name="wk", bufs=2) as wk,
              tc.tile_pool(name="psI", bufs=3, space="PSUM") as psI,
              tc.tile_pool(name="psF", bufs=1, space="PSUM") as psF,
              tc.tile_pool(name="psT", bufs=1, space="PSUM") as psT,
              tc.tile_pool(name="psH", bufs=2, space="PSUM") as psH):

            # ---- persistent loads ----
            featT = bp.tile([128, SPC * KCH * C], f32r, tag="featT")
            MT = bp.tile([128, SPC * KCH * NPQ], f32r, tag="MT")
            for s in range(SPC):
                for k in range(KCH):
                    nc.sync.dma_start(
                        featT[:, (s * KCH + k) * C:(s * KCH + k + 1) * C],
                        d["featT"][s, k * 128:(k + 1) * 128, :])
                    nc.sync.dma_start(
                        MT[:, (s * KCH + k) * NPQ:(s * KCH + k + 1) * NPQ],
                        d["MT"][s, k * 128:(k + 1) * 128, :])

            def cload(name, shape, dt=f32r):
                t = cp.tile(list(shape), dt, tag=name, name=name + "_sb")
                nc.sync.dma_start(t[:], d[name][:])
                return t
            ones_sb = cload("ones", (1, 128))
            fcb_sb = cload("fcbias", (1, HID))
            id_sb = cload("ident", (128, 128), f32)
            maskC = cload("maskC", (S * 4, MROWS), f32)
            maskP = cload("maskP", (S, MROWS), f32)
            maskL = cload("maskL", (NLOCR, MROWS), f32)
            maskG = cload("maskG", (NGRPR, GRP_SZ * SPC * N_GRP), f32)
            cbias = cload("cbias", (S * 4, 1), f32)
            pbias = cload("pbias", (S, 1), f32)
            lbias = cload("lbias", (NLOCR, 1), f32)
            gbias = cload("gbias", (NGRPR, 1), f32)
            predT = cload("predT", (S * 4, SPC), f32)
            whT = cload("whT", (S * 4, SPC), f32)

            cwT = cp.tile([128, CCH * S * 4], f32r, tag="cwT")
            pwT = cp.tile([128, CCH * S], f32r, tag="pwT")
            lwT = cp.tile([128, CCH * NLOCR], f32r, tag="lwT")
            gwT = cp.tile([128, GRP_SZ * CCH * NGRPR], f32r, tag="gwT")
            for hc in range(CCH):
                nc.sync.dma_start(cwT[:, hc * S * 4:(hc + 1) * S * 4],
                                  d["cwT"][hc * 128:(hc + 1) * 128, :])
                nc.sync.dma_start(pwT[:, hc * S:(hc + 1) * S],
                                  d["pwT"][hc * 128:(hc + 1) * 128, :])
                nc.sync.dma_start(lwT[:, hc * NLOCR:(hc + 1) * NLOCR],
                                  d["lwT"][hc * 128:(hc + 1) * 128, :])
                for m in range(GRP_SZ):
                    nc.sync.dma_start(
                        gwT[:, (m * CCH + hc) * NGRPR:(m * CCH + hc + 1) * NGRPR],
                        d["gwT"][m, hc * 128:(hc + 1) * 128, :])

            # ---- interleaved interp + FC, channel-chunk major ----
            pooled = [bp.tile([128, SPC * NPQ], f32r, tag=f"pooled{cc}",
                              name=f"pooled{cc}")
                      for cc in range(CCH)]
            fps = psF.tile([MROWS, HID], f32, tag="psF", name="psFt")
            # bias via K=1 rank-1 matmul (starts the PSUM accumulation group)
            nc.tensor.matmul(fps[:], ones_sb[:, :MROWS], fcb_sb[:],
                             start=True, stop=False)
            for cc in range(CCH):
                # interp for this channel chunk (both samples)
                for s in range(SPC):
                    for (n0, nn) in NCHUNKS:
                        pt = psI.tile([128, nn], f32, tag="psI", name="ptI")
                        for k in range(KCH):
                            base = (s * KCH + k)
                            nc.tensor.matmul(
                                pt[:],
                                featT[:, base * C + cc * 128: base * C + cc * 128 + 128],
                                MT[:, base * NPQ + n0: base * NPQ + n0 + nn],
                                start=(k == 0), stop=(k == KCH - 1))
                        nc.vector.tensor_copy(
                            pooled[cc][:, s * NPQ + n0: s * NPQ + n0 + nn], pt[:])
                # FC k-chunks for this channel chunk (weight stream)
                for (b0, nb) in WBATCH:
                    wt = wp.tile([128, WB * HID], f32r, tag="wst", name="wt")
                    nc.sync.dma_start(
                        wt[:, :nb * HID],
                        d["Wst"][cc, b0:b0 + nb].rearrange("pq r h -> r pq h"))
                    for i in range(nb):
                        pq = b0 + i
                        nc.tensor.matmul(
                            fps[:],
                            pooled[cc][:, pq::PQ],
                            wt[:, i * HID:(i + 1) * HID],
                            start=False,
                            stop=(cc == CCH - 1 and pq == PQ - 1))

            feats = wk.tile([MROWS, HID], f32, tag="feats")
            nc.vector.tensor_relu(feats[:], fps[:])

            # ---- transpose feats -> featsT (h-part, (s,n)-free) ----
            featsT = wk.tile([128, CCH * MROWS], f32r, tag="featsT")
            for hc in range(CCH):
                tp = psT.tile([128, MROWS], f32, tag="psT", name="tpT")
                nc.tensor.transpose(tp[:], feats[:, hc * 128:(hc + 1) * 128],
                                    id_sb[:MROWS, :MROWS])
                nc.vector.tensor_copy(featsT[:, hc * MROWS:(hc + 1) * MROWS], tp[:])

            # ---- heads (matmul + mask + segment-reduce) ----
            def head_mm(wtile, nrows, wstride, name):
                hp = psH.tile([nrows, MROWS], f32, tag="psH", name=name)
                for hc in range(CCH):
                    nc.tensor.matmul(
                        hp[:], wtile[:, hc * wstride: hc * wstride + nrows],
                        featsT[:, hc * MROWS:(hc + 1) * MROWS],
                        start=(hc == 0), stop=(hc == CCH - 1))
                return hp

            def mask_reduce(hp, nrows, mask, tagn):
                mskd = wk.tile([nrows, MROWS], f32, tag="m" + tagn,
                               name="m" + tagn)
                nc.vector.tensor_mul(mskd[:], hp[:], mask[:])
                red = wk.tile([nrows, SPC], f32, tag="r" + tagn,
                              name="r" + tagn)
                nc.vector.reduce_sum(
                    red[:], mskd[:].rearrange("p (s n) -> p s n", s=SPC),
                    axis=mybir.AxisListType.X)
                return red

            hpC = head_mm(cwT, S * 4, S * 4, "hpC")
            redC = mask_reduce(hpC, S * 4, maskC, "C")
            nc.vector.tensor_scalar_add(redC[:], redC[:], cbias[:])
            nc.vector.tensor_mul(redC[:], redC[:], whT[:])
            nc.vector.tensor_add(redC[:], redC[:], predT[:])
            nc.sync.dma_start(d["o_ref"][:], redC[:])

            hpP = head_mm(pwT, S, S, "hpP")
            redP = mask_reduce(hpP, S, maskP, "P")
            nc.vector.tensor_scalar_add(redP[:], redP[:], pbias[:])
            nc.sync.dma_start(d["o_pres"][:], redP[:])

            hpL = head_mm(lwT, NLOCR, NLOCR, "hpL")
            redL = mask_reduce(hpL, NLOCR, maskL, "L")
            nc.vector.tensor_scalar_add(redL[:], redL[:], lbias[:])
            nc.sync.dma_start(d["o_loc"][:], redL[:])

            gp = psH.tile([NGRPR, GRP_SZ * SPC * N_GRP], f32, tag="psH",
                          name="gp")
            for m in range(GRP_SZ):
                for hc in range(CCH):
                    rhsv = (featsT[:, hc * MROWS:(hc + 1) * MROWS]
                            .rearrange("p (s n) -> p s n", s=SPC)
                            [:, :, m:m + (N_GRP - 1) * GRP_SZ + 1:GRP_SZ])
                    nc.tensor.matmul(
                        gp[:, m * SPC * N_GRP:(m + 1) * SPC * N_GRP],
                        gwT[:, (m * CCH + hc) * NGRPR:(m * CCH + hc + 1) * NGRPR],
                        rhsv,
                        start=(hc == 0), stop=(hc == CCH - 1))
            mskdG = wk.tile([NGRPR, GRP_SZ * SPC * N_GRP], f32, tag="mG")
            nc.vector.tensor_mul(mskdG[:], gp[:], maskG[:])
            r1 = wk.tile([NGRPR, GRP_SZ * SPC], f32, tag="r1G")
            nc.vector.reduce_sum(
                r1[:], mskdG[:].rearrange("p (m s g) -> p m s g", s=SPC, g=N_GRP),
                axis=mybir.AxisListType.X)
            redG = wk.tile([NGRPR, SPC], f32, tag="rG")
            nc.vector.reduce_sum(
                redG[:], r1[:].rearrange("p (m s) -> p s m", s=SPC),
                axis=mybir.AxisListType.X)
            nc.vector.tensor_scalar_add(redG[:], redG[:], gbias[:])
            nc.sync.dma_start(d["o_grp"][:], redG[:])

    nc.compile()
    return nc


def kernel(**inputs) -> tuple:
    global _BUILT
    if _BUILT is None:
        _BUILT = _build_program()
    nc = _BUILT

    lf = np.asarray(inputs["local_features"], np.float32)
    coords = np.asarray(inputs["pred_bbox_coords"], np.float32)
    Wfc = np.asarray(inputs["roi_fc_W"], np.float32)
    fcb = np.asarray(inputs["roi_fc_b"], np.float32)
    coords_W = np.asarray(inputs["coords_W"], np.float32)
    coords_b = np.asarray(inputs["coords_b"], np.float32)
    pres_W = np.asarray(inputs["pres_W"], np.float32)
    pres_b = np.asarray(inputs["pres_b"], np.float32)
    loc_W = np.asarray(inputs["loc_W"], np.float32)
    loc_b = np.asarray(inputs["loc_b"], np.float32)
    grp_W = np.asarray(inputs["grp_W"], np.float32)
    grp_b = np.asarray(inputs["grp_b"], np.float32)
    loc_idx = np.asarray(inputs["loc_idx"], np.int64)
    grp_idx = np.asarray(inputs["grp_idx"], np.int64)

    # ---- host prep ----
    Qy, Qx = _interp_mats(coords)
    MT = np.einsum("bnph,bnqw->bhwnpq", Qy, Qx).reshape(B, H * H, NPQ)
    featT = lf.transpose(0, 2, 3, 1).reshape(B, H * H, C)

    # Wst[cc, pq, ci, j] = Wfc[j, (cc*128+ci)*PQ + pq]
    Wst = np.ascontiguousarray(
        Wfc.reshape(HID, CCH, 128, PQ).transpose(1, 3, 2, 0))

    cwT = coords_W.transpose(2, 0, 1).reshape(HID, S * 4)
    pwT = pres_W.T
    lwT = loc_W.transpose(2, 0, 1).reshape(HID, N_LOC * N_LOC_LAB)
    gw4 = grp_W.reshape(N_GRP, N_GRP_LAB, GRP_SZ, HID)
    gwT = np.stack([gw4[:, :, m, :].transpose(2, 0, 1)
                    .reshape(HID, N_GRP * N_GRP_LAB) for m in range(GRP_SZ)])

    maskC = np.zeros((S * 4, MROWS), np.float32)
    for n in range(S):
        for o in range(4):
            for s in range(SPC):
                maskC[n * 4 + o, s * S + n] = 1.0
    maskP = np.zeros((S, MROWS), np.float32)
    for n in range(S):
        for s in range(SPC):
            maskP[n, s * S + n] = 1.0
    maskL = np.zeros((N_LOC * N_LOC_LAB, MROWS), np.float32)
    for l in range(N_LOC):
        for o in range(N_LOC_LAB):
            for s in range(SPC):
                maskL[l * N_LOC_LAB + o, s * S + int(loc_idx[l])] = 1.0
    exp_gidx = np.arange(N_GRP * GRP_SZ).reshape(N_GRP, GRP_SZ)
    assert np.array_equal(grp_idx, exp_gidx), "grp_idx pattern unsupported"
    maskG = np.zeros((N_GRP * N_GRP_LAB, GRP_SZ * SPC * N_GRP), np.float32)
    for g in range(N_GRP):
        for o in range(N_GRP_LAB):
            for m in range(GRP_SZ):
                for s in range(SPC):
                    maskG[g * N_GRP_LAB + o, m * SPC * N_GRP + s * N_GRP + g] = 1.0

    shared = {
        "Wst": _r(Wst), "fcbias": _r(fcb.reshape(1, HID)),
        "ones": _r(np.ones((1, 128), np.float32)),
        "ident": np.eye(128, dtype=np.float32),
        "cwT": _r(cwT), "pwT": _r(pwT), "lwT": _r(lwT), "gwT": _r(gwT),
        "maskC": maskC, "maskP": maskP, "maskL": maskL, "maskG": maskG,
        "cbias": coords_b.reshape(S * 4, 1).astype(np.float32),
        "pbias": pres_b.reshape(S, 1).astype(np.float32),
        "lbias": loc_b.reshape(N_LOC * N_LOC_LAB, 1).astype(np.float32),
        "gbias": grp_b.reshape(N_GRP * N_GRP_LAB, 1).astype(np.float32),
    }

    w = coords[..., 2] - coords[..., 0]
    h = coords[..., 3] - coords[..., 1]
    wh = np.stack([w, h, w, h], axis=-1)  # (B,S,4)

    in_maps = []
    for k in range(NCORES):
        sl = slice(k * SPC, (k + 1) * SPC)
        m = dict(shared)
        m["featT"] = _r(featT[sl])
        m["MT"] = _r(MT[sl])
        m["predT"] = np.ascontiguousarray(
            coords[sl].transpose(1, 2, 0).reshape(S * 4, SPC))
        m["whT"] = np.ascontiguousarray(
            wh[sl].transpose(1, 2, 0).reshape(S * 4, SPC))
        in_maps.append(m)

    res = bass_utils.run_bass_kernel_spmd(nc, in_maps, core_ids=list(range(NCORES)))
    kernel.last_result = res

    refined = np.zeros((B, S, 4), np.float32)
    presence = np.zeros((B, S), np.float32)
    mlc = np.zeros((B, N_LOC * N_LOC_LAB + N_GRP * N_GRP_LAB), np.float32)
    for k in range(NCORES):
        r = res.results[k]
        for s in range(SPC):
            b = k * SPC + s
            refined[b] = r["o_ref"][:, s].reshape(S, 4)
            presence[b] = r["o_pres"][:, s]
            mlc[b, :N_LOC * N_LOC_LAB] = r["o_loc"][:, s]
            mlc[b, N_LOC * N_LOC_LAB:] = r["o_grp"][:, s]
    return refined, presence, mlc
